# revision 3
# baseline (speedup 1.0000x reference)
"""LogScale (histogram_binning) Trainium2 kernel.

out[..., :n_lin]          = linear interp of x at fixed pairs      (PE matmul)
out[..., n_lin:n_lin+n_c] = Catmull-Rom cubic interp of x          (PE matmul)
out[..., n_lin+n_c:]      = max over windows of (x + tri_weights)  (DVE add + reduce_max)

Sharding: pure data parallel over the flattened (32*512) leading dim,
8 cores x 2048 rows each.
"""

import math
import sys

import numpy as np

for _p in ("/opt/trn_rl_repo",):
    if _p not in sys.path:
        sys.path.insert(0, _p)

from contextlib import ExitStack

import concourse.bass as bass
import concourse.tile as tile
from concourse import mybir
from concourse.bass_utils import run_bass_kernel_spmd
from concourse.vector_clock import ScopedClock

F32 = mybir.dt.float32

# --- workaround: this walrus build only accepts ONE sem wait per instruction ---

def _split_dab(self, tick_clock, wait_clock):
    nc = self.nc
    nops = [nc.sync.nop(nofuse=True) for _ in range(32)]
    drain_inst = nc.sync.drain()
    wait_clock.add_sem_waits(drain_inst.ins,
                             ScopedClock({None: tick_clock.global_clock}))
    si = drain_inst.ins.sync_info
    if si is not None and len(si.on_wait) > 1:
        waits = list(si.on_wait)
        for nop_b, wv in zip(nops, waits[:-1]):
            nop_b.ins.sync_info = mybir.SyncInfo(on_wait=[wv], on_update=[])
        drain_inst.ins.sync_info = mybir.SyncInfo(on_wait=[waits[-1]],
                                                  on_update=[])
    nc.all_engine_barrier()
    popped = nc._tile_sem_poison_stack.pop()
    assert popped is self._sem_poison
    nc.clear_and_free_semaphores(list(self.sems.allocated().values()))
    nc.all_engine_barrier()


tile.TileContext._drain_and_barrier = _split_dab


def _legalize_waits(nc):
    """Split any instruction carrying >1 sem wait into preceding same-engine
    1-wait NoOps (this walrus encodes at most one wait per instruction)."""
    nid = [0]
    for fn in nc.m.functions:
        for bb in fn.blocks:
            insts = list(bb.instructions)
            out = []
            changed = False
            for inst in insts:
                si = inst.sync_info
                waits = list(si.on_wait) if si is not None else []
                if len(waits) > 1:
                    changed = True
                    for wv in waits[:-1]:
                        nop = mybir.InstNoOp(
                            name=f"waitsplit-{nid[0]}", ins=[], outs=[])
                        nid[0] += 1
                        nop.engine = inst.engine
                        nop.sync_info = mybir.SyncInfo(on_wait=[wv],
                                                       on_update=[])
                        out.append(nop)
                    inst.sync_info = mybir.SyncInfo(
                        on_wait=[waits[-1]], on_update=list(si.on_update))
                out.append(inst)
            if changed:
                try:
                    bb.instructions = out
                except (AttributeError, TypeError):
                    cur = bb.instructions
                    if cur is not insts and hasattr(cur, "clear"):
                        cur.clear()
                        cur.extend(out)
                    else:
                        raise
                assert len(list(bb.instructions)) == len(out), \
                    "block instruction list mutation did not stick"

N_CORES = 8
P = 128          # partitions / rows per tile
XPAD = 2112      # padded x-tile width (>= 2049 + max segment overreach)
KCH = 3          # 128-bin K-chunks used by the lin/cubic matmul (bins 0..383)
SEG_OV = 116     # DVE per-segment overhead (2 ops x ~58 cycles) for the DP


def _tri_segments(starts, ends, n_tri):
    """DP: split windows into segments with affine cover (stride c, width W),
    minimizing 2*G*W + overhead per segment."""
    INF = float("inf")
    ncost = [INF] * (n_tri + 1)
    ncost[0] = 0.0
    choice = [None] * (n_tri + 1)
    for b in range(1, n_tri + 1):
        for a in range(max(0, b - 80), b):
            G = b - a
            d = np.arange(G)
            best = None
            for c in range(0, 16):
                off_lo = int((starts[a:b] - c * d).min())
                W = int((ends[a:b] - c * d).max()) - off_lo
                if off_lo < 0:
                    continue
                if off_lo + c * (G - 1) + W > XPAD:
                    continue
                cost = G * W
                if best is None or cost < best[0]:
                    best = (cost, c, off_lo, W)
            if best is None:
                continue
            tot = ncost[a] + SEG_OV + 2 * best[0]
            if tot < ncost[b]:
                ncost[b] = tot
                choice[b] = (a, best[1], best[2], best[3])
    segs = []
    b = n_tri
    while b > 0:
        a, c, base, W = choice[b]
        segs.append((a, b, c, base, W))
        b = a
    segs.reverse()
    return segs


def _build_program(n_rows, n_in, n_out, n_lc, nnzp, segs):
    nc = bass.Bass()
    x_ext = nc.declare_dram_parameter("x", [n_rows, n_in], F32, isOutput=False)
    mm_ext = nc.declare_dram_parameter("mmat", [KCH * P, n_lc], F32, isOutput=False)
    wr_ext = nc.declare_dram_parameter("wrep", [1, nnzp], F32, isOutput=False)
    id_ext = nc.declare_dram_parameter("ident", [P, P], F32, isOutput=False)
    out_ext = nc.declare_dram_parameter("out", [n_rows, n_out], F32, isOutput=True)

    ntiles = n_rows // P
    assert n_rows % P == 0

    with ExitStack() as ctx:
        tc = ctx.enter_context(tile.TileContext(nc))
        singles = ctx.enter_context(tc.tile_pool(name="singles", bufs=1))
        xpool = ctx.enter_context(tc.tile_pool(name="xp", bufs=3))
        xwpool = ctx.enter_context(tc.tile_pool(name="xw", bufs=2))
        opool = ctx.enter_context(tc.tile_pool(name="op", bufs=3))
        xtpool = ctx.enter_context(tc.tile_pool(name="xt", bufs=2))
        ptpool = ctx.enter_context(tc.tile_pool(name="pt", bufs=2, space="PSUM"))
        popool = ctx.enter_context(tc.tile_pool(name="po", bufs=2, space="PSUM"))

        # constants
        mm_s = singles.tile([P, KCH, n_lc], F32)
        nc.sync.dma_start(out=mm_s, in_=mm_ext[:].rearrange("(k p) n -> p k n", p=P))
        wr_s = singles.tile([P, nnzp], F32)
        wsrc = wr_ext[:]
        wbc = bass.AP(tensor=wsrc.tensor, offset=wsrc.offset,
                      ap=[[0, P], list(wsrc.ap[-1])])
        nc.gpsimd.dma_start(out=wr_s, in_=wbc)
        id_s = singles.tile([P, P], F32)
        nc.sync.dma_start(out=id_s, in_=id_ext[:])

        for it in range(ntiles):
            r0 = it * P
            xt = xpool.tile([P, XPAD], F32)
            nc.sync.dma_start(out=xt[:, 0:1024], in_=x_ext[r0:r0 + P, 0:1024])
            nc.sync.dma_start(out=xt[:, 1024:n_in], in_=x_ext[r0:r0 + P, 1024:n_in])
            nc.gpsimd.memset(xt[:, n_in:XPAD], 0.0)

            # ---- lin + cubic on PE ----
            pt = ptpool.tile([P, KCH, P], F32)
            for k in range(KCH):
                nc.tensor.transpose(pt[:, k, :], xt[:, k * P:(k + 1) * P], id_s)
            xts = xtpool.tile([P, KCH, P], F32)
            nc.scalar.copy(xts, pt)
            ot = opool.tile([P, n_out], F32)
            for n0 in range(0, n_lc, 512):
                n1 = min(n0 + 512, n_lc)
                po = popool.tile([P, 512], F32, tag="po")
                for k in range(KCH):
                    nc.tensor.matmul(po[:, 0:n1 - n0], lhsT=xts[:, k, :],
                                     rhs=mm_s[:, k, n0:n1],
                                     start=(k == 0), stop=(k == KCH - 1))
                nc.scalar.copy(ot[:, n0:n1], po[:, 0:n1 - n0])

            # ---- tri on DVE ----
            xw = xwpool.tile([P, nnzp], F32)
            off = 0
            for (a, b, c, base, W) in segs:
                G = b - a
                sl = xt[:, base:base + W]
                src = bass.AP(tensor=sl.tensor, offset=sl.offset,
                              ap=[list(sl.ap[0]), [c, G], [1, W]])
                dst = xw[:, off:off + G * W].rearrange("p (g w) -> p g w", w=W)
                wseg = wr_s[:, off:off + G * W].rearrange("p (g w) -> p g w", w=W)
                nc.vector.tensor_add(dst, src, wseg)
                off += G * W
            off = 0
            for (a, b, c, base, W) in segs:
                G = b - a
                nc.vector.reduce_max(
                    out=ot[:, n_lc + a:n_lc + b],
                    in_=xw[:, off:off + G * W].rearrange("p (g w) -> p g w", w=W),
                    axis=mybir.AxisListType.X)
                off += G * W

            nc.sync.dma_start(out=out_ext[r0:r0 + P, :], in_=ot)
    _legalize_waits(nc)
    return nc


def _prepare(fraction_linear, fraction_cubic, triangular_weights, linear_pair_idx):
    flin = np.asarray(fraction_linear, dtype=np.float32)
    fcub = np.asarray(fraction_cubic, dtype=np.float32)
    w = np.asarray(triangular_weights, dtype=np.float32)
    pidx = np.asarray(linear_pair_idx, dtype=np.int64)

    n_lin = flin.shape[0]
    n_cub = fcub.shape[0]
    n_tri, n_in = w.shape
    n_lc = n_lin + n_cub

    # lin/cubic coefficient matrix
    mmat = np.zeros((KCH * P, n_lc), dtype=np.float32)
    p0 = pidx[:n_lin]
    mmat[p0, np.arange(n_lin)] += (1.0 - flin).astype(np.float32)
    mmat[p0 + 1, np.arange(n_lin)] += flin
    i0 = np.floor(fcub).astype(np.int64)
    f = (fcub - i0.astype(np.float32)).astype(np.float32)
    cm1 = 0.5 * (-f + 2 * f * f - f ** 3)
    c0 = 1.0 - 2.5 * f * f + 1.5 * f ** 3
    c1 = 0.5 * f + 2 * f * f - 1.5 * f ** 3
    c2 = 0.5 * (f ** 3 - f * f)
    cols = n_lin + np.arange(n_cub)
    for kk, cf in zip((-1, 0, 1, 2), (cm1, c0, c1, c2)):
        mmat[i0 + kk, cols] += cf.astype(np.float32)
    assert int(i0.max()) + 2 < KCH * P and int(p0.max()) + 1 < KCH * P

    # tri windows
    finite = np.isfinite(w)
    starts = np.array([np.flatnonzero(finite[j])[0] for j in range(n_tri)])
    ends = np.array([np.flatnonzero(finite[j])[-1] + 1 for j in range(n_tri)])
    segs = _tri_segments(starts, ends, n_tri)
    nnzp = sum((b - a) * W for a, b, c, base, W in segs)

    wflat = np.full(nnzp, -1e30, dtype=np.float32)
    off = 0
    for (a, b, c, base, W) in segs:
        for j in range(a, b):
            oj = base + c * (j - a)
            for k in range(W):
                bin_ = oj + k
                if bin_ < n_in and finite[j, bin_]:
                    wflat[off + (j - a) * W + k] = w[j, bin_]
        off += (b - a) * W

    return mmat, wflat, segs, nnzp, n_lin, n_cub, n_tri, n_lc


_CACHE = {}


def kernel(x, fraction_linear, fraction_cubic, triangular_weights, linear_pair_idx):
    x = np.asarray(x, dtype=np.float32)
    B, T, n_in = x.shape
    flat = np.ascontiguousarray(x.reshape(-1, n_in))
    rows = flat.shape[0]
    assert rows % N_CORES == 0
    R = rows // N_CORES

    mmat, wflat, segs, nnzp, n_lin, n_cub, n_tri, n_lc = _prepare(
        fraction_linear, fraction_cubic, triangular_weights, linear_pair_idx)
    n_out = n_lc + n_tri

    key = (R, n_in, n_out, n_lc, nnzp, tuple(segs))
    if key not in _CACHE:
        _CACHE[key] = _build_program(R, n_in, n_out, n_lc, nnzp, segs)
    nc = _CACHE[key]

    ident = np.eye(P, dtype=np.float32)
    wrep = wflat[None, :]
    in_maps = [
        {"x": np.ascontiguousarray(flat[i * R:(i + 1) * R]),
         "mmat": mmat, "wrep": wrep, "ident": ident}
        for i in range(N_CORES)
    ]
    res = run_bass_kernel_spmd(nc, in_maps, list(range(N_CORES)))
    out = np.concatenate([res.results[i]["out"] for i in range(N_CORES)], axis=0)
    return out.reshape(B, T, n_out).astype(np.float32)
